# revision 4
# baseline (speedup 1.0000x reference)
"""Trainium2 Bass kernel for the raw-reshape RoPE attention problem.

Math structure (verified against the reference):
  The reference reshapes [B, N, H*D] -> [B, H, N, D] with a *raw* reshape
  (no transpose).  Viewing the projection [2048, 1024] as [32768, 64],
  head h covers rows [h*2048, (h+1)*2048) -- i.e. head h only sees input
  tokens [h*128, (h+1)*128), and the final output rows [h*128, (h+1)*128)
  depend only on head h.  So the 32 (b, h) pairs are fully independent:
  4 pairs per NeuronCore, no collectives.

  Within a head we use the row permutation j' = s*128 + t (orig j = t*16+s,
  s = weight block, t = token).  Softmax/attention are invariant to a
  consistent row permutation of q/k/v; it makes every on-device layout
  change a contiguous [64, 128] block copy.

  RoPE rotates pair (2i, 2i+1) of each 64-channel block by the angle of
  position j-1 (row j=0 unrotated).  We pre-permute wq/wk rows on the host
  so even channels land in [s*64, s*64+32) and odd in [s*64+32, s*64+64),
  making the on-device rotation pure contiguous-block arithmetic.  The
  permutation cancels inside the q.k dot products.

Per (b, h) pair on device (all matmuls bf16, fp32 accumulation):
  Qp/Kp/Vp = Xp @ Wt           (lhsT = x.T blocks, host-pretransposed)
  rope(Qp), rope(Kp) on DVE    -> bf16 [128 tok, 1024 chan]
  PE-transpose + [64,128] block copies -> q2T/k2T [64 d, 2048 j'] bf16
  for j-half, key-chunk c: scoresT = k2T_c.T @ q2T; E = exp(s/8) (ScalarE)
  out_accT[65, j] += [V_c | 1].T @ E   (row 64 accumulates softmax sums)
  normalize via VectorE reciprocal + GPSIMD partition_broadcast
  final = out_tok @ wo.T; DMA out.
"""

import numpy as np
import ml_dtypes

N_CORES = 8
B, N, DIM = 2, 2048, 1024
H, HD = 16, 64
PAIRS_PER_CORE = 4
BF16 = ml_dtypes.bfloat16

_CACHE = {}


def _rope_tables():
    """cos/sin tables [128 t, 16 s, 32 i]; (t=0, s=0) is the unrotated row."""
    inv = 1.0 / (10000.0 ** (np.arange(0, HD, 2, dtype=np.float32) / HD))
    pos = np.arange(128 * 16, dtype=np.float32).reshape(128, 16) - 1.0  # j-1
    ang = pos[:, :, None] * inv[None, None, :]          # [128, 16, 32]
    c = np.cos(ang).astype(np.float32)
    s = np.sin(ang).astype(np.float32)
    c[0, 0, :] = 1.0
    s[0, 0, :] = 0.0
    return c, s


def _chan_perm():
    """c = s*64 + 2i + par -> c' = s*64 + par*32 + i."""
    perm = np.zeros(DIM, np.int64)
    for s in range(16):
        i = np.arange(32)
        perm[s * 64 + i] = s * 64 + 2 * i
        perm[s * 64 + 32 + i] = s * 64 + 2 * i + 1
    return perm


def _build_nc():
    import concourse.mybir as mybir
    import concourse.tile as tile
    from concourse import bacc
    from concourse.masks import make_identity

    dt = mybir.dt
    AF = mybir.ActivationFunctionType

    nc = bacc.Bacc("TRN2", target_bir_lowering=False, debug=False,
                   num_devices=N_CORES)

    xt_d = nc.declare_dram_parameter("xt", [PAIRS_PER_CORE, 128, 8, 128],
                                     dt.bfloat16, isOutput=False)
    w_d = {}
    for name in ("wq", "wk", "wv", "wo"):
        w_d[name] = nc.declare_dram_parameter(name + "t", [128, 8, 1024],
                                              dt.bfloat16, isOutput=False)
    rc_d = nc.declare_dram_parameter("ropec", [128, 16, 32], dt.float32,
                                     isOutput=False)
    rs_d = nc.declare_dram_parameter("ropes", [128, 16, 32], dt.float32,
                                     isOutput=False)
    out_d = nc.declare_dram_parameter("out", [PAIRS_PER_CORE, 128, 1024],
                                      dt.float32, isOutput=True)

    with tile.TileContext(nc) as tc:
        with (
            tc.tile_pool(name="wts", bufs=1) as wts,
            tc.tile_pool(name="const", bufs=1) as constp,
            tc.tile_pool(name="xin", bufs=2) as xin,
            tc.tile_pool(name="tmp", bufs=2) as tmpp,
            tc.tile_pool(name="qkr", bufs=2) as qkrp,
            tc.tile_pool(name="vsb", bufs=2) as vsbp,
            tc.tile_pool(name="qkt", bufs=2) as qktp,
            tc.tile_pool(name="esb", bufs=4) as esbp,
            tc.tile_pool(name="rsb", bufs=2) as rsbp,
            tc.tile_pool(name="osb", bufs=2) as osbp,
            tc.tile_pool(name="psA", bufs=1, space="PSUM") as psA,
            tc.tile_pool(name="psS", bufs=2, space="PSUM") as psS,
            tc.tile_pool(name="psW", bufs=1, space="PSUM") as psW,
        ):
            # resident constants / weights
            w_sb = {}
            for name in ("wq", "wk", "wv", "wo"):
                t = wts.tile([128, 8, 1024], dt.bfloat16, tag=name)
                nc.sync.dma_start(t[:], w_d[name][:])
                w_sb[name] = t
            rc = constp.tile([128, 16, 32], dt.float32, tag="rc")
            rs = constp.tile([128, 16, 32], dt.float32, tag="rs")
            nc.sync.dma_start(rc[:], rc_d[:])
            nc.sync.dma_start(rs[:], rs_d[:])
            ident = constp.tile([128, 128], dt.bfloat16, tag="id")
            make_identity(nc, ident[:])

            for p in range(PAIRS_PER_CORE):
                xs = xin.tile([128, 8, 128], dt.bfloat16, tag="xt")
                nc.sync.dma_start(xs[:], xt_d[p])

                # ---- projections (+rope for q/k, evac for v) ----
                qr = qkrp.tile([128, 16, 2, 32], dt.bfloat16, tag="qr")
                kr = qkrp.tile([128, 16, 2, 32], dt.bfloat16, tag="kr")
                vsb = vsbp.tile([128, 16, 65], dt.bfloat16, tag="v")
                nc.vector.memset(vsb[:, :, 64:65], 1.0)

                for tname, dst in (("wq", qr), ("wk", kr), ("wv", vsb)):
                    if tname == "wv":
                        pp = psW.tile([128, 16, 64], dt.float32, tag="w")
                        for nt in range(2):
                            for kk in range(8):
                                nc.tensor.matmul(
                                    pp[:, nt * 8:(nt + 1) * 8, :],
                                    xs[:, kk, :],
                                    w_sb[tname][:, kk, nt * 512:(nt + 1) * 512],
                                    start=(kk == 0), stop=(kk == 7))
                        nc.vector.tensor_copy(vsb[:, :, 0:64], pp[:])
                        continue
                    pp = psW.tile([128, 16, 2, 32], dt.float32, tag="w")
                    for nt in range(2):
                        for kk in range(8):
                            nc.tensor.matmul(
                                pp[:, nt * 8:(nt + 1) * 8, :, :],
                                xs[:, kk, :],
                                w_sb[tname][:, kk, nt * 512:(nt + 1) * 512],
                                start=(kk == 0), stop=(kk == 7))
                    xe, xo = pp[:, :, 0, :], pp[:, :, 1, :]
                    t1 = tmpp.tile([128, 16, 32], dt.float32, tag="t1")
                    t2 = tmpp.tile([128, 16, 32], dt.float32, tag="t2")
                    nc.vector.tensor_mul(t1[:], xe, rc[:])
                    nc.vector.tensor_mul(t2[:], xo, rs[:])
                    nc.vector.tensor_sub(dst[:, :, 0, :], t1[:], t2[:])
                    t3 = tmpp.tile([128, 16, 32], dt.float32, tag="t1")
                    t4 = tmpp.tile([128, 16, 32], dt.float32, tag="t2")
                    nc.vector.tensor_mul(t3[:], xe, rs[:])
                    nc.vector.tensor_mul(t4[:], xo, rc[:])
                    nc.vector.tensor_add(dst[:, :, 1, :], t3[:], t4[:])

                # ---- transpose to q2T/k2T [64 d, 16 s, 128 t] ----
                q2t = qktp.tile([64, 16, 128], dt.bfloat16, tag="q2t")
                k2t = qktp.tile([64, 16, 128], dt.bfloat16, tag="k2t")
                for src, dstT in ((qr, q2t), (kr, k2t)):
                    for kk in range(8):
                        tp = psW.tile([128, 128], dt.bfloat16, tag="w")
                        nc.tensor.transpose(
                            tp[:], src[:, 2 * kk:2 * kk + 2, :, :], ident[:])
                        for sub in range(2):
                            s = 2 * kk + sub
                            nc.vector.tensor_copy(
                                dstT[:, s, :], tp[sub * 64:(sub + 1) * 64, :])

                # ---- attention (key-chunk streaming, j in halves) ----
                otok = osbp.tile([128, 8, 128], dt.bfloat16, tag="otok")
                for jh in range(2):
                    acc = psA.tile([65, 8, 128], dt.float32, tag="acc")
                    for c in range(16):
                        e = esbp.tile([128, 1024], dt.bfloat16, tag="e")
                        sct = psS.tile([128, 1024], dt.float32, tag="s")
                        for jt in range(2):
                            s0 = jh * 8 + jt * 4
                            nc.tensor.matmul(
                                sct[:, jt * 512:(jt + 1) * 512],
                                k2t[:, c, :],
                                q2t[:, s0:s0 + 4, :],
                                start=True, stop=True)
                        nc.scalar.activation(e[:], sct[:], AF.Exp, scale=0.125)
                        for jt in range(2):
                            nc.tensor.matmul(
                                acc[:, jt * 4:(jt + 1) * 4, :],
                                vsb[:, c, :],
                                e[:, jt * 512:(jt + 1) * 512],
                                start=(c == 0), stop=(c == 15),
                                skip_group_check=True)

                    # ---- normalize + assemble out_tok.T ----
                    srow = rsbp.tile([1, 8, 128], dt.float32, tag="srow")
                    nc.vector.tensor_copy(srow[:], acc[64:65, :, :])
                    rsb = rsbp.tile([1, 8, 128], dt.float32, tag="r")
                    # custom-DVE op requires base partition 0 on its input
                    nc.vector.reciprocal_approx_fast(out=rsb[:], in_=srow[:])
                    rbc = rsbp.tile([64, 8, 128], dt.float32, tag="rbc")
                    nc.gpsimd.partition_broadcast(rbc[:], rsb[:])
                    for sl in range(8):
                        s = jh * 8 + sl
                        nc.vector.tensor_mul(
                            otok[(s % 2) * 64:(s % 2) * 64 + 64, s // 2, :],
                            acc[0:64, sl, :],
                            rbc[:, sl, :])

                # ---- output projection ----
                fin = psW.tile([128, 1024], dt.float32, tag="w")
                for nt in range(2):
                    for kk in range(8):
                        nc.tensor.matmul(
                            fin[:, nt * 512:(nt + 1) * 512],
                            otok[:, kk, :],
                            w_sb["wo"][:, kk, nt * 512:(nt + 1) * 512],
                            start=(kk == 0), stop=(kk == 7))
                osb = osbp.tile([128, 1024], dt.float32, tag="osb")
                nc.vector.tensor_copy(osb[:], fin[:])
                nc.sync.dma_start(out_d[p], osb[:])

    nc.compile()
    return nc


def _get_nc():
    if "nc" not in _CACHE:
        _CACHE["nc"] = _build_nc()
    return _CACHE["nc"]


def _prep_inputs(x, wq, wk, wv, wo):
    perm = _chan_perm()
    ropec, ropes = _rope_tables()

    def wt(w):
        # [out_chan, dim] -> transposed, partition-major [128, 8, 1024]
        return np.ascontiguousarray(
            w.T.reshape(8, 128, DIM).transpose(1, 0, 2)).astype(BF16)

    wqt = wt(wq[perm, :])
    wkt = wt(wk[perm, :])
    wvt = wt(wv)
    wot = wt(wo)

    in_maps = []
    for core in range(N_CORES):
        xts = np.empty((PAIRS_PER_CORE, 128, 8, 128), BF16)
        for pl in range(PAIRS_PER_CORE):
            pg = core * PAIRS_PER_CORE + pl
            b, h = pg // H, pg % H
            X = x[b, h * 128:(h + 1) * 128, :]      # [128 tok, 1024]
            xts[pl] = np.ascontiguousarray(
                X.T.reshape(8, 128, 128).transpose(1, 0, 2)).astype(BF16)
        in_maps.append({
            "xt": xts,
            "wqt": wqt, "wkt": wkt, "wvt": wvt, "wot": wot,
            "ropec": ropec, "ropes": ropes,
        })
    return in_maps


def run_sharded(x, wq, wk, wv, wo, trace=False, **run_kwargs):
    """Build + run on 8 cores; returns (full_output, BassKernelResults)."""
    from concourse.bass_utils import run_bass_kernel_spmd

    nc = _get_nc()
    in_maps = _prep_inputs(np.asarray(x, np.float32), np.asarray(wq, np.float32),
                           np.asarray(wk, np.float32), np.asarray(wv, np.float32),
                           np.asarray(wo, np.float32))
    res = run_bass_kernel_spmd(nc, in_maps, core_ids=list(range(N_CORES)),
                               trace=trace, **run_kwargs)
    out = np.empty((B, N, DIM), np.float32)
    for core in range(N_CORES):
        o = np.asarray(res.results[core]["out"], np.float32)
        for pl in range(PAIRS_PER_CORE):
            pg = core * PAIRS_PER_CORE + pl
            b, h = pg // H, pg % H
            out[b, h * 128:(h + 1) * 128, :] = o[pl]
    return out, res


def kernel(x, wq, wk, wv, wo):
    out, _ = run_sharded(x, wq, wk, wv, wo, trace=False)
    return out


# revision 7
# speedup vs baseline: 1.1202x; 1.1202x over previous
"""Trainium2 Bass kernel for the raw-reshape RoPE attention problem.

Math structure (verified against the reference):
  The reference reshapes [B, N, H*D] -> [B, H, N, D] with a *raw* reshape
  (no transpose).  Viewing the projection [2048, 1024] as [32768, 64],
  head h covers rows [h*2048, (h+1)*2048) -- i.e. head h only sees input
  tokens [h*128, (h+1)*128), and the final output rows [h*128, (h+1)*128)
  depend only on head h.  So the 32 (b, h) pairs are fully independent:
  4 pairs per NeuronCore, no collectives.

  Within a head we use the row permutation j' = s*128 + t (orig j = t*16+s,
  s = weight block, t = token).  Softmax/attention are invariant to a
  consistent row permutation of q/k/v; it makes every on-device layout
  change a contiguous [64, 128] block copy.

  RoPE rotates pair (2i, 2i+1) of each 64-channel block by the angle of
  position j-1 (row j=0 unrotated).  We pre-permute wq/wk rows on the host
  so even channels land in [s*64, s*64+32) and odd in [s*64+32, s*64+64),
  making the on-device rotation pure contiguous-block arithmetic.  The
  permutation cancels inside the q.k dot products.

Per (b, h) pair on device (all matmuls bf16, fp32 accumulation):
  Qp/Kp/Vp = Xp @ Wt           (lhsT = x.T blocks, host-pretransposed)
  rope(Qp), rope(Kp) on DVE    -> bf16 [128 tok, 1024 chan]
  PE-transpose + [64,128] block copies -> q2T/k2T [64 d, 2048 j'] bf16
  for j-half, key-chunk c: scoresT = k2T_c.T @ q2T; E = exp(s/8) (ScalarE)
  out_accT[65, j] += [V_c | 1].T @ E   (row 64 accumulates softmax sums)
  normalize via VectorE reciprocal + GPSIMD partition_broadcast
  final = out_tok @ wo.T; DMA out.
"""

import numpy as np
import ml_dtypes

N_CORES = 8
B, N, DIM = 2, 2048, 1024
H, HD = 16, 64
PAIRS_PER_CORE = 4
BF16 = ml_dtypes.bfloat16

_CACHE = {}


def _rope_tables():
    """cos/sin tables [128 t, 16 s, 32 i]; (t=0, s=0) is the unrotated row."""
    inv = 1.0 / (10000.0 ** (np.arange(0, HD, 2, dtype=np.float32) / HD))
    pos = np.arange(128 * 16, dtype=np.float32).reshape(128, 16) - 1.0  # j-1
    ang = pos[:, :, None] * inv[None, None, :]          # [128, 16, 32]
    c = np.cos(ang).astype(np.float32)
    s = np.sin(ang).astype(np.float32)
    c[0, 0, :] = 1.0
    s[0, 0, :] = 0.0
    return c, s


def _chan_perm():
    """c = s*64 + 2i + par -> c' = s*64 + par*32 + i."""
    perm = np.zeros(DIM, np.int64)
    for s in range(16):
        i = np.arange(32)
        perm[s * 64 + i] = s * 64 + 2 * i
        perm[s * 64 + 32 + i] = s * 64 + 2 * i + 1
    return perm


def _build_nc():
    import concourse.mybir as mybir
    import concourse.tile as tile
    from concourse import bacc
    from concourse.masks import make_identity

    dt = mybir.dt
    AF = mybir.ActivationFunctionType

    nc = bacc.Bacc("TRN2", target_bir_lowering=False, debug=False,
                   num_devices=N_CORES)

    xt_d = nc.declare_dram_parameter("xt", [PAIRS_PER_CORE, 128, 8, 128],
                                     dt.bfloat16, isOutput=False)
    w_d = {}
    for name in ("wq", "wk", "wv", "wo"):
        w_d[name] = nc.declare_dram_parameter(name + "t", [128, 8, 1024],
                                              dt.bfloat16, isOutput=False)
    rc_d = nc.declare_dram_parameter("ropec", [128, 16, 32], dt.float32,
                                     isOutput=False)
    rs_d = nc.declare_dram_parameter("ropes", [128, 16, 32], dt.float32,
                                     isOutput=False)
    out_d = nc.declare_dram_parameter("out", [PAIRS_PER_CORE, 128, 1024],
                                      dt.float32, isOutput=True)

    with tile.TileContext(nc) as tc:
        with (
            tc.tile_pool(name="wts", bufs=1) as wts,
            tc.tile_pool(name="const", bufs=1) as constp,
            tc.tile_pool(name="xin", bufs=2) as xin,
            tc.tile_pool(name="tmp", bufs=2) as tmpp,
            tc.tile_pool(name="qkr", bufs=2) as qkrp,
            tc.tile_pool(name="vsb", bufs=2) as vsbp,
            tc.tile_pool(name="qkt", bufs=2) as qktp,
            tc.tile_pool(name="esb", bufs=4) as esbp,
            tc.tile_pool(name="rsb", bufs=2) as rsbp,
            tc.tile_pool(name="osb", bufs=2) as osbp,
            tc.tile_pool(name="psA", bufs=1, space="PSUM") as psA,
            tc.tile_pool(name="psS", bufs=2, space="PSUM") as psS,
            tc.tile_pool(name="psW", bufs=1, space="PSUM") as psW,
        ):
            # resident constants / weights
            w_sb = {}
            for name in ("wq", "wk", "wv", "wo"):
                t = wts.tile([128, 8, 1024], dt.bfloat16, tag=name)
                nc.sync.dma_start(t[:], w_d[name][:])
                w_sb[name] = t
            rc = constp.tile([128, 16, 32], dt.float32, tag="rc")
            rs = constp.tile([128, 16, 32], dt.float32, tag="rs")
            nc.sync.dma_start(rc[:], rc_d[:])
            nc.sync.dma_start(rs[:], rs_d[:])
            ident = constp.tile([128, 128], dt.bfloat16, tag="id")
            make_identity(nc, ident[:])

            # persistent attention operands, zero-padded to full PE-array
            # shapes (keeps HAM activity high -> 2.4 GHz during attention)
            q2t = qktp.tile([128, 16, 128], dt.bfloat16, tag="q2t")
            k2t = qktp.tile([128, 16, 128], dt.bfloat16, tag="k2t")
            vsb = vsbp.tile([128, 16, 128], dt.bfloat16, tag="v")
            nc.gpsimd.memset(q2t[64:128, :, :], 0.0)
            nc.gpsimd.memset(k2t[64:128, :, :], 0.0)
            nc.gpsimd.memset(vsb[:, :, 65:128], 0.0)
            nc.vector.memset(vsb[:, :, 64:65], 1.0)

            for p in range(PAIRS_PER_CORE):
                xs = xin.tile([128, 8, 128], dt.bfloat16, tag="xt")
                nc.sync.dma_start(xs[:], xt_d[p])

                # ---- projections (+rope for q/k, evac for v) ----
                qr = qkrp.tile([128, 16, 2, 32], dt.bfloat16, tag="qr")
                kr = qkrp.tile([128, 16, 2, 32], dt.bfloat16, tag="kr")

                for tname, dst in (("wq", qr), ("wk", kr), ("wv", vsb)):
                    if tname == "wv":
                        pp = psW.tile([128, 16, 64], dt.float32, tag="w")
                        for nt in range(2):
                            for kk in range(8):
                                nc.tensor.matmul(
                                    pp[:, nt * 8:(nt + 1) * 8, :],
                                    xs[:, kk, :],
                                    w_sb[tname][:, kk, nt * 512:(nt + 1) * 512],
                                    start=(kk == 0), stop=(kk == 7))
                        nc.vector.tensor_copy(vsb[:, :, 0:64], pp[:])
                        continue
                    pp = psW.tile([128, 16, 2, 32], dt.float32, tag="w")
                    for nt in range(2):
                        for kk in range(8):
                            nc.tensor.matmul(
                                pp[:, nt * 8:(nt + 1) * 8, :, :],
                                xs[:, kk, :],
                                w_sb[tname][:, kk, nt * 512:(nt + 1) * 512],
                                start=(kk == 0), stop=(kk == 7))
                    xe, xo = pp[:, :, 0, :], pp[:, :, 1, :]
                    t1 = tmpp.tile([128, 16, 32], dt.float32, tag="t1")
                    t2 = tmpp.tile([128, 16, 32], dt.float32, tag="t2")
                    nc.vector.tensor_mul(t1[:], xe, rc[:])
                    nc.vector.tensor_mul(t2[:], xo, rs[:])
                    nc.vector.tensor_sub(dst[:, :, 0, :], t1[:], t2[:])
                    t3 = tmpp.tile([128, 16, 32], dt.float32, tag="t1")
                    t4 = tmpp.tile([128, 16, 32], dt.float32, tag="t2")
                    nc.vector.tensor_mul(t3[:], xe, rs[:])
                    nc.vector.tensor_mul(t4[:], xo, rc[:])
                    nc.vector.tensor_add(dst[:, :, 1, :], t3[:], t4[:])

                # ---- transpose to q2T/k2T rows 0:64 ----
                for src_, dstT in ((qr, q2t), (kr, k2t)):
                    for kk in range(8):
                        tp = psW.tile([128, 128], dt.bfloat16, tag="w")
                        nc.tensor.transpose(
                            tp[:], src_[:, 2 * kk:2 * kk + 2, :, :], ident[:])
                        for sub in range(2):
                            s = 2 * kk + sub
                            nc.vector.tensor_copy(
                                dstT[0:64, s, :],
                                tp[sub * 64:(sub + 1) * 64, :])

                # ---- attention (key-chunk streaming, j in halves) ----
                otok = osbp.tile([128, 8, 128], dt.bfloat16, tag="otok")
                for jh in range(2):
                    acc = psA.tile([128, 8, 128], dt.float32, tag="acc")
                    for c in range(16):
                        e = esbp.tile([128, 1024], dt.bfloat16, tag="e")
                        sct = psS.tile([128, 1024], dt.float32, tag="s")
                        for jt in range(2):
                            s0 = jh * 8 + jt * 4
                            nc.tensor.matmul(
                                sct[:, jt * 512:(jt + 1) * 512],
                                k2t[:, c, :],
                                q2t[:, s0:s0 + 4, :],
                                start=True, stop=True)
                        nc.scalar.activation(e[:], sct[:], AF.Exp, scale=0.125)
                        for jt in range(2):
                            nc.tensor.matmul(
                                acc[:, jt * 4:(jt + 1) * 4, :],
                                vsb[:, c, :],
                                e[:, jt * 512:(jt + 1) * 512],
                                start=(c == 0), stop=(c == 15),
                                skip_group_check=True)

                    # ---- normalize + assemble out_tok.T ----
                    srow = rsbp.tile([1, 8, 128], dt.float32, tag="srow")
                    nc.vector.tensor_copy(srow[:], acc[64:65, :, :])
                    rsb = rsbp.tile([1, 8, 128], dt.float32, tag="r")
                    # custom-DVE op requires base partition 0 on its input
                    nc.vector.reciprocal_approx_fast(out=rsb[:], in_=srow[:])
                    rbc = rsbp.tile([64, 8, 128], dt.float32, tag="rbc")
                    nc.gpsimd.partition_broadcast(rbc[:], rsb[:])
                    for sl in range(8):
                        s = jh * 8 + sl
                        nc.vector.tensor_mul(
                            otok[(s % 2) * 64:(s % 2) * 64 + 64, s // 2, :],
                            acc[0:64, sl, :],
                            rbc[:, sl, :])

                # ---- output projection ----
                fin = psW.tile([128, 1024], dt.float32, tag="w")
                for nt in range(2):
                    for kk in range(8):
                        nc.tensor.matmul(
                            fin[:, nt * 512:(nt + 1) * 512],
                            otok[:, kk, :],
                            w_sb["wo"][:, kk, nt * 512:(nt + 1) * 512],
                            start=(kk == 0), stop=(kk == 7))
                osb = osbp.tile([128, 1024], dt.float32, tag="osb")
                nc.vector.tensor_copy(osb[:], fin[:])
                nc.sync.dma_start(out_d[p], osb[:])

    nc.compile()
    return nc


def _get_nc():
    if "nc" not in _CACHE:
        _CACHE["nc"] = _build_nc()
    return _CACHE["nc"]


def _prep_inputs(x, wq, wk, wv, wo):
    perm = _chan_perm()
    ropec, ropes = _rope_tables()

    def wt(w):
        # [out_chan, dim] -> transposed, partition-major [128, 8, 1024]
        return np.ascontiguousarray(
            w.T.reshape(8, 128, DIM).transpose(1, 0, 2)).astype(BF16)

    wqt = wt(wq[perm, :])
    wkt = wt(wk[perm, :])
    wvt = wt(wv)
    wot = wt(wo)

    in_maps = []
    for core in range(N_CORES):
        xts = np.empty((PAIRS_PER_CORE, 128, 8, 128), BF16)
        for pl in range(PAIRS_PER_CORE):
            pg = core * PAIRS_PER_CORE + pl
            b, h = pg // H, pg % H
            X = x[b, h * 128:(h + 1) * 128, :]      # [128 tok, 1024]
            xts[pl] = np.ascontiguousarray(
                X.T.reshape(8, 128, 128).transpose(1, 0, 2)).astype(BF16)
        in_maps.append({
            "xt": xts,
            "wqt": wqt, "wkt": wkt, "wvt": wvt, "wot": wot,
            "ropec": ropec, "ropes": ropes,
        })
    return in_maps


def run_sharded(x, wq, wk, wv, wo, trace=False, **run_kwargs):
    """Build + run on 8 cores; returns (full_output, BassKernelResults)."""
    from concourse.bass_utils import run_bass_kernel_spmd

    nc = _get_nc()
    in_maps = _prep_inputs(np.asarray(x, np.float32), np.asarray(wq, np.float32),
                           np.asarray(wk, np.float32), np.asarray(wv, np.float32),
                           np.asarray(wo, np.float32))
    res = run_bass_kernel_spmd(nc, in_maps, core_ids=list(range(N_CORES)),
                               trace=trace, **run_kwargs)
    out = np.empty((B, N, DIM), np.float32)
    for core in range(N_CORES):
        o = np.asarray(res.results[core]["out"], np.float32)
        for pl in range(PAIRS_PER_CORE):
            pg = core * PAIRS_PER_CORE + pl
            b, h = pg // H, pg % H
            out[b, h * 128:(h + 1) * 128, :] = o[pl]
    return out, res


def kernel(x, wq, wk, wv, wo):
    out, _ = run_sharded(x, wq, wk, wv, wo, trace=False)
    return out
